# revision 1
# baseline (speedup 1.0000x reference)
"""BinaryFactoredLinear Trainium2 kernel.

Computes out = ((x * s2) @ sign(V)) @ sign(U).T * s1 + bias for
x [4, 4096, 4096] f32, factors [4096, 128] / [4096] — token-sharded
across 8 NeuronCores (2048 tokens each), run SPMD via
run_bass_kernel_spmd.

Host prep (exact f32 math, negligible vs HW time): x2 = x * s2 (same
op order as the reference), then x2 is split into xhi = bf16(x2) and
xlo = bf16(x2 - xhi) — together they carry ~16 mantissa bits, and the
sign matrices are +-1 so bf16 weights are exact. Each core's token
shard is pre-transposed and pre-tiled into contiguous [128, T] blocks
so every DMA is a contiguous 1 MiB transfer with the contraction dim
on SBUF partitions (no on-chip transposes, no on-chip dtype
conversions). The core writes its output transposed as contiguous
[nt, no, 128, T] blocks which the host reassembles.

Per-core pipeline (tokens tiled by T=512, all matmuls N=512 bf16):
  stage 1: z1T[r=128, T] += V_sign_k.T @ xhi_k + V_sign_k.T @ xlo_k
           (32 k-chunks accumulated in one PSUM bank)
  z1 split: DVE re-splits z1 (f32 PSUM) into bf16 hi/lo
  stage 2: outT[o*128:(o+1)*128, T] = U_sign_o @ [z1hi; z1lo]
  epilogue: ScalarE activation(Identity, scale=s1, bias=bias) — both
            per-partition APs — during the PSUM -> SBUF copy.

End-to-end rel err vs the f32 reference: ~3.5e-6 (HW-verified).
Other modes kept for experiments: f32 (exact, 4 cyc/row), f32r
(1 cyc/row, ~1.3e-4 on HW), bf16 (~2.4e-3), bf16x2 (on-chip hi/lo
split, same numerics as bf16x2h but extra ACT/DVE conversion load).
"""

import os
from contextlib import ExitStack

import numpy as np

import concourse.bacc as bacc
import concourse.mybir as mybir
import concourse.tile as tile
from concourse.bass_utils import run_bass_kernel_spmd

F32 = mybir.dt.float32
F32R = mybir.dt.float32r
BF16 = mybir.dt.bfloat16

B, S, D_IN, D_OUT, R = 4, 4096, 4096, 4096, 128
N_CORES = 8
TOKENS = B * S
TOK_PER_CORE = TOKENS // N_CORES

MODE = os.environ.get("BFL_MODE", "bf16x2h")
T_TILE = int(os.environ.get("BFL_T_TILE", "512"))
DMA_GROUP = int(os.environ.get("BFL_DMA_GROUP", "4"))
EPI = os.environ.get("BFL_EPI", "act")
LO_ENG = os.environ.get("BFL_LO_ENG", "dve")
XBUFS = int(os.environ.get("BFL_XBUFS", "5"))
LAYOUT = os.environ.get("BFL_LAYOUT", "std")


def build_nc(mode=MODE, d_in=D_IN, d_out=D_OUT, r=R, tok=TOK_PER_CORE,
             t_tile=T_TILE, loop=1, dma_group=DMA_GROUP, epi=EPI,
             lo_eng=LO_ENG, xbufs=XBUFS, layout=LAYOUT, probe="full",
             odma=os.environ.get("BFL_ODMA", "spread"), obufs=3, opbufs=4):
    assert d_in % 128 == 0 and d_out % 128 == 0 and tok % t_tile == 0
    assert r == 128 and t_tile <= 512
    nk, no, nt = d_in // 128, d_out // 128, tok // t_tile
    g = dma_group
    assert nk % g == 0 and no % g == 0

    if mode == "f32":
        xdt = wdt = F32
    elif mode == "f32r":
        xdt = wdt = F32R
    elif mode == "bf16x2h":
        xdt = wdt = BF16
    else:
        xdt, wdt = F32, BF16

    nc = bacc.Bacc("TRN2", target_bir_lowering=False, debug=False)

    if layout == "fat":
        xt = nc.dram_tensor("xt", [nt, nk // g, 128, g, t_tile], xdt,
                            kind="ExternalInput")
        outt = nc.dram_tensor("outt", [nt, no // g, 128, g, t_tile], F32,
                              kind="ExternalOutput")
    else:
        xt = nc.dram_tensor("xt", [nt, nk, 128, t_tile], xdt,
                            kind="ExternalInput")
        outt = nc.dram_tensor("outt", [nt, no, 128, t_tile], F32,
                              kind="ExternalOutput")
    if mode == "bf16x2h":
        assert layout == "std"
        xt2 = nc.dram_tensor("xt2", [nt, nk, 128, t_tile], BF16,
                             kind="ExternalInput")
    w1 = nc.dram_tensor("w1", [128, nk, r], wdt, kind="ExternalInput")
    w2 = nc.dram_tensor("w2", [r, d_out], wdt, kind="ExternalInput")
    s1c = nc.dram_tensor("s1c", [128, no], F32, kind="ExternalInput")
    biasc = nc.dram_tensor("biasc", [128, no], F32, kind="ExternalInput")

    Copy = mybir.ActivationFunctionType.Copy
    Ident = mybir.ActivationFunctionType.Identity
    sub = mybir.AluOpType.subtract
    mult = mybir.AluOpType.mult
    add = mybir.AluOpType.add
    lo_iface = nc.gpsimd if lo_eng == "pool" else nc.vector
    if odma == "spread":
        _rr = [0]

        def _dma():
            _rr[0] += 1
            return nc.sync if _rr[0] % 2 else nc.gpsimd
        in_dma = out_dma = lambda: _dma()
    else:
        out_iface = nc.gpsimd if odma == "pool" else nc.sync
        in_dma = lambda: nc.sync
        out_dma = lambda: out_iface

    with tile.TileContext(nc) as tc, ExitStack() as ctx:
        const = ctx.enter_context(tc.tile_pool(name="const", bufs=1))
        xpool = ctx.enter_context(tc.tile_pool(name="x", bufs=xbufs))
        z1s = ctx.enter_context(tc.tile_pool(name="z1s", bufs=2))
        osb = ctx.enter_context(tc.tile_pool(name="osb", bufs=obufs))
        z1pool = ctx.enter_context(
            tc.tile_pool(name="z1p", bufs=2, space="PSUM"))
        opsum = ctx.enter_context(
            tc.tile_pool(name="opsum", bufs=opbufs, space="PSUM"))
        if mode in ("bf16", "bf16x2"):
            hpool = ctx.enter_context(tc.tile_pool(name="hi", bufs=2 * xbufs))
        if mode == "bf16x2":
            lpool = ctx.enter_context(tc.tile_pool(name="lo", bufs=2 * xbufs))

        w1_sb = const.tile([128, nk, r], wdt)
        nc.sync.dma_start(w1_sb[:], w1.ap())
        w2_sb = const.tile([128, d_out], wdt)
        nc.sync.dma_start(w2_sb[:], w2.ap())
        s1_sb = const.tile([128, no], F32)
        nc.sync.dma_start(s1_sb[:], s1c.ap())
        b_sb = const.tile([128, no], F32)
        nc.sync.dma_start(b_sb[:], biasc.ap())

        if loop > 1:
            loop_cm = tc.For_i(
                0, loop, 1,
                hint_engines=(mybir.EngineType.PE, mybir.EngineType.DVE,
                              mybir.EngineType.Activation,
                              mybir.EngineType.SP))
            ctx.enter_context(loop_cm)

        for t in range(nt):
            z1p = z1pool.tile([128, t_tile], F32)
            xg, xg2 = {}, {}
            for kg in range(nk // g):
                xk = xpool.tile([128, g, t_tile], xdt)
                if probe != "nodma":
                    if layout == "fat":
                        in_dma().dma_start(xk[:], xt.ap()[t, kg])
                    else:
                        in_dma().dma_start(
                            xk[:], xt.ap()[t, kg * g:(kg + 1) * g].rearrange(
                                "g p s -> p g s"))
                xg[kg] = xk
                if mode == "bf16x2h":
                    xk2 = xpool.tile([128, g, t_tile], BF16, tag="xk2",
                                     name="xk2")
                    if probe != "nodma":
                        in_dma().dma_start(
                            xk2[:],
                            xt2.ap()[t, kg * g:(kg + 1) * g].rearrange(
                                "g p s -> p g s"))
                    xg2[kg] = xk2
            for k in range(nk):
                xk = xg[k // g][:, k % g, :]
                first, last = k == 0, k == nk - 1
                if mode == "bf16x2h":
                    xk2 = xg2[k // g][:, k % g, :]
                    nc.tensor.matmul(z1p[:], w1_sb[:, k, :], xk,
                                     start=first, stop=False)
                    nc.tensor.matmul(z1p[:], w1_sb[:, k, :], xk2,
                                     start=False, stop=last)
                elif mode in ("bf16", "bf16x2"):
                    hi = hpool.tile([128, t_tile], BF16)
                    nc.scalar.activation(hi[:], xk, Copy)
                    if mode == "bf16x2":
                        lo = lpool.tile([128, t_tile], BF16)
                        lo_iface.tensor_tensor(lo[:], xk, hi[:], sub)
                        nc.tensor.matmul(z1p[:], w1_sb[:, k, :], hi[:],
                                         start=first, stop=False)
                        nc.tensor.matmul(z1p[:], w1_sb[:, k, :], lo[:],
                                         start=False, stop=last)
                    else:
                        nc.tensor.matmul(z1p[:], w1_sb[:, k, :], hi[:],
                                         start=first, stop=last)
                else:
                    nc.tensor.matmul(z1p[:], w1_sb[:, k, :], xk,
                                     start=first, stop=last)

            if mode in ("bf16", "bf16x2", "bf16x2h"):
                z1hi = z1s.tile([128, t_tile], BF16, tag="z1hi")
                nc.vector.tensor_copy(z1hi[:], z1p[:])
                movers = [z1hi]
                if mode in ("bf16x2", "bf16x2h"):
                    z1lo = z1s.tile([128, t_tile], BF16, tag="z1lo")
                    nc.vector.tensor_tensor(z1lo[:], z1p[:], z1hi[:], sub)
                    movers.append(z1lo)
            else:
                z1f = z1s.tile([128, t_tile], xdt, tag="z1f")
                nc.vector.tensor_copy(z1f[:], z1p[:])
                movers = [z1f]

            for og in range(no // g):
                ob = osb.tile([128, g, t_tile], F32)
                for oi in range(g):
                    o = og * g + oi
                    op = opsum.tile([128, t_tile], F32)
                    for i, mv in enumerate(movers):
                        nc.tensor.matmul(
                            op[:], w2_sb[:, o * 128:(o + 1) * 128], mv[:],
                            start=(i == 0), stop=(i == len(movers) - 1))
                    if epi == "act":
                        nc.scalar.activation(ob[:, oi, :], op[:], Ident,
                                             bias=b_sb[:, o:o + 1],
                                             scale=s1_sb[:, o:o + 1])
                    else:
                        nc.vector.tensor_scalar(ob[:, oi, :], op[:],
                                                s1_sb[:, o:o + 1],
                                                b_sb[:, o:o + 1], mult, add)
                if probe != "nodma":
                    if layout == "fat":
                        out_dma().dma_start(outt.ap()[t, og], ob[:])
                    else:
                        out_dma().dma_start(
                            outt.ap()[t, og * g:(og + 1) * g].rearrange(
                                "g p s -> p g s"), ob[:])

    nc.compile()
    return nc


def prep_inputs(x, U_latent, V_latent, s1, s2, bias, mode=MODE,
                n_cores=N_CORES, t_tile=T_TILE, layout=LAYOUT,
                dma_group=DMA_GROUP):
    """Host-side prep: fold s2 into x, sign + cast factors, shard tokens."""
    import ml_dtypes

    tokens = x.shape[0] * x.shape[1] if x.ndim == 3 else x.shape[0]
    d_in = x.shape[-1]
    tok_pc = tokens // n_cores
    nt, nk = tok_pc // t_tile, d_in // 128
    g = dma_group

    x2 = x.reshape(tokens, d_in) * s2[None, :]
    w1 = np.sign(V_latent).astype(np.float32)
    # pack [d_in, r] -> [128, nk, r] so the SBUF upload is contiguous
    w1 = np.ascontiguousarray(
        w1.reshape(nk, 128, -1).transpose(1, 0, 2))
    w2 = np.ascontiguousarray(np.sign(U_latent).astype(np.float32).T)
    if mode in ("bf16", "bf16x2", "bf16x2h"):
        w1 = w1.astype(ml_dtypes.bfloat16)
        w2 = w2.astype(ml_dtypes.bfloat16)
    if mode == "bf16x2h":
        xhi = x2.astype(ml_dtypes.bfloat16)
        xlo = (x2 - xhi.astype(np.float32)).astype(ml_dtypes.bfloat16)
    no = w2.shape[1] // 128
    s1c = np.ascontiguousarray(s1.reshape(no, 128).T)
    biasc = np.ascontiguousarray(bias.reshape(no, 128).T)

    def tilefmt(arr2d, c):
        xs = arr2d[c * tok_pc:(c + 1) * tok_pc, :]
        if layout == "fat":
            # [nt, T, nk/g, g, 128] -> [nt, nk/g, 128, g, T]:
            # per partition a contiguous g*T run
            return np.ascontiguousarray(
                xs.reshape(nt, t_tile, nk // g, g, 128).transpose(
                    0, 2, 4, 3, 1))
        # [nt, T, nk, 128] -> [nt, nk, 128, T]
        return np.ascontiguousarray(
            xs.reshape(nt, t_tile, nk, 128).transpose(0, 2, 3, 1))

    in_maps = []
    for c in range(n_cores):
        m = {"w1": w1, "w2": w2, "s1c": s1c, "biasc": biasc}
        if mode == "bf16x2h":
            m["xt"] = tilefmt(xhi, c)
            m["xt2"] = tilefmt(xlo, c)
        else:
            m["xt"] = tilefmt(x2, c)
        in_maps.append(m)
    return in_maps


def gather_out(results, n_cores=N_CORES, t_tile=T_TILE, layout=LAYOUT,
               dma_group=DMA_GROUP):
    out = np.empty((TOKENS, D_OUT), np.float32)
    for c in range(n_cores):
        ot = results[c]["outt"]
        if layout == "fat":
            # [nt, no/g, 128, g, T] -> [tok_pc, d_out]
            shard = ot.transpose(0, 4, 1, 3, 2).reshape(TOK_PER_CORE, D_OUT)
        else:
            # [nt, no, 128, T] -> [tok_pc, d_out]
            shard = ot.transpose(0, 3, 1, 2).reshape(TOK_PER_CORE, D_OUT)
        out[c * TOK_PER_CORE:(c + 1) * TOK_PER_CORE, :] = shard
    return out.reshape(B, S, D_OUT)


_NC_CACHE = {}


def run(inputs, mode=MODE, trace=False):
    if mode not in _NC_CACHE:
        _NC_CACHE[mode] = build_nc(mode=mode)
    nc = _NC_CACHE[mode]
    in_maps = prep_inputs(**inputs, mode=mode)
    res = run_bass_kernel_spmd(nc, in_maps, list(range(N_CORES)),
                               trace=trace)
    return gather_out(res.results), res


def kernel(**inputs):
    inputs = {k: np.asarray(v) for k, v in inputs.items()}
    out, _ = run(inputs)
    return out



# revision 9
# speedup vs baseline: 2.2121x; 2.2121x over previous
"""BinaryFactoredLinear Trainium2 kernel.

Computes out = ((x * s2) @ sign(V)) @ sign(U).T * s1 + bias for
x [4, 4096, 4096] f32, factors [4096, 128] / [4096] — token-sharded
across 8 NeuronCores (2048 tokens each), run SPMD via
run_bass_kernel_spmd.

Host prep (exact f32 math, negligible vs HW time): x2 = x * s2 (same
op order as the reference), then x2 is split into xhi = bf16(x2) and
xlo = bf16(x2 - xhi) — together they carry ~16 mantissa bits, and the
sign matrices are +-1 so bf16 weights are exact. Each core's token
shard is pre-transposed and pre-tiled into contiguous [128, T] blocks
so every DMA is a contiguous 1 MiB transfer with the contraction dim
on SBUF partitions (no on-chip transposes, no on-chip dtype
conversions). The core writes its output transposed as contiguous
[nt, no, 128, T] blocks which the host reassembles.

Per-core pipeline (tokens tiled by T=512, all matmuls N=512 bf16):
  stage 1: z1T[r=128, T] += V_sign_k.T @ xhi_k + V_sign_k.T @ xlo_k
           (32 k-chunks accumulated in one PSUM bank)
  z1 split: DVE re-splits z1 (f32 PSUM) into bf16 hi/lo
  stage 2: outT[o*128:(o+1)*128, T] = U_sign_o @ [z1hi; z1lo]
  epilogue: ScalarE activation(Identity, scale=s1, bias=bias) — both
            per-partition APs — during the PSUM -> SBUF copy.

End-to-end rel err vs the f32 reference: ~3.5e-6 (HW-verified).
Other modes kept for experiments: f32 (exact, 4 cyc/row), f32r
(1 cyc/row, ~1.3e-4 on HW), bf16 (~2.4e-3), bf16x2 (on-chip hi/lo
split, same numerics as bf16x2h but extra ACT/DVE conversion load).
"""

import os
from contextlib import ExitStack

import numpy as np

import concourse.bacc as bacc
import concourse.mybir as mybir
import concourse.tile as tile
from concourse.bass_utils import run_bass_kernel_spmd

F32 = mybir.dt.float32
F32R = mybir.dt.float32r
BF16 = mybir.dt.bfloat16
F8 = mybir.dt.float8e4

B, S, D_IN, D_OUT, R = 4, 4096, 4096, 4096, 128
N_CORES = 8
TOKENS = B * S
TOK_PER_CORE = TOKENS // N_CORES

MODE = os.environ.get("BFL_MODE", "bf16x2h")
T_TILE = int(os.environ.get("BFL_T_TILE", "512"))
DMA_GROUP = int(os.environ.get("BFL_DMA_GROUP", "4"))
EPI = os.environ.get("BFL_EPI", "act")
LO_ENG = os.environ.get("BFL_LO_ENG", "dve")
XBUFS = int(os.environ.get("BFL_XBUFS", "5"))
LAYOUT = os.environ.get("BFL_LAYOUT", "std")


def build_nc(mode=MODE, d_in=D_IN, d_out=D_OUT, r=R, tok=TOK_PER_CORE,
             t_tile=T_TILE, loop=1, dma_group=DMA_GROUP, epi=EPI,
             lo_eng=LO_ENG, xbufs=XBUFS, layout=LAYOUT, probe="full",
             odma=os.environ.get("BFL_ODMA", "spread"), obufs=3, opbufs=4):
    assert d_in % 128 == 0 and d_out % 128 == 0 and tok % t_tile == 0
    assert r == 128 and t_tile <= 512
    nk, no, nt = d_in // 128, d_out // 128, tok // t_tile
    g = dma_group
    assert nk % g == 0 and no % g == 0

    if mode == "f32":
        xdt = wdt = F32
    elif mode == "f32r":
        xdt = wdt = F32R
    elif mode in ("bf16x2h", "bf16s"):
        xdt = wdt = BF16
    elif mode == "fp8dr":
        xdt = wdt = F8
    else:
        xdt, wdt = F32, BF16
    out_dt = BF16 if mode in ("bf16s", "fp8dr") else F32
    DR = mybir.MatmulPerfMode.DoubleRow

    nc = bacc.Bacc("TRN2", target_bir_lowering=False, debug=False)

    if layout == "fat":
        xt = nc.dram_tensor("xt", [nt, nk // g, 128, g, t_tile], xdt,
                            kind="ExternalInput")
        outt = nc.dram_tensor("outt", [nt, no // g, 128, g, t_tile], out_dt,
                              kind="ExternalOutput")
    elif mode == "fp8dr":
        xt = nc.dram_tensor("xt", [nt, nk, 128, 2, t_tile], F8,
                            kind="ExternalInput")
        outt = nc.dram_tensor("outt", [nt, no, 128, t_tile], out_dt,
                              kind="ExternalOutput")
    else:
        xt = nc.dram_tensor("xt", [nt, nk, 128, t_tile], xdt,
                            kind="ExternalInput")
        outt = nc.dram_tensor("outt", [nt, no, 128, t_tile], out_dt,
                              kind="ExternalOutput")
    if mode == "bf16x2h":
        assert layout == "std"
        xt2 = nc.dram_tensor("xt2", [nt, nk, 128, t_tile], BF16,
                             kind="ExternalInput")
    if mode == "fp8dr":
        w1 = nc.dram_tensor("w1", [128, nk, 2, r], F8, kind="ExternalInput")
        w2 = nc.dram_tensor("w2", [r, 2, d_out], F8, kind="ExternalInput")
    else:
        w1 = nc.dram_tensor("w1", [128, nk, r], wdt, kind="ExternalInput")
        w2 = nc.dram_tensor("w2", [r, d_out], wdt, kind="ExternalInput")
    s1c = nc.dram_tensor("s1c", [128, no], F32, kind="ExternalInput")
    biasc = nc.dram_tensor("biasc", [128, no], F32, kind="ExternalInput")

    Copy = mybir.ActivationFunctionType.Copy
    Ident = mybir.ActivationFunctionType.Identity
    sub = mybir.AluOpType.subtract
    mult = mybir.AluOpType.mult
    add = mybir.AluOpType.add
    lo_iface = nc.gpsimd if lo_eng == "pool" else nc.vector
    if odma == "spread":
        _rr = [0]

        def _dma():
            _rr[0] += 1
            return nc.sync if _rr[0] % 2 else nc.gpsimd
        in_dma = out_dma = lambda: _dma()
    else:
        out_iface = nc.gpsimd if odma == "pool" else nc.sync
        in_dma = lambda: nc.sync
        out_dma = lambda: out_iface

    with tile.TileContext(nc) as tc, ExitStack() as ctx:
        const = ctx.enter_context(tc.tile_pool(name="const", bufs=1))
        xpool = ctx.enter_context(tc.tile_pool(name="x", bufs=xbufs))
        z1s = ctx.enter_context(tc.tile_pool(name="z1s", bufs=2))
        osb = ctx.enter_context(tc.tile_pool(name="osb", bufs=obufs))
        z1pool = ctx.enter_context(
            tc.tile_pool(name="z1p", bufs=2, space="PSUM"))
        opsum = ctx.enter_context(
            tc.tile_pool(name="opsum", bufs=opbufs, space="PSUM"))
        if mode in ("bf16", "bf16x2"):
            hpool = ctx.enter_context(tc.tile_pool(name="hi", bufs=2 * xbufs))
        if mode == "bf16x2":
            lpool = ctx.enter_context(tc.tile_pool(name="lo", bufs=2 * xbufs))

        if mode == "fp8dr":
            w1_sb = const.tile([128, nk, 2, r], F8)
            w2_sb = const.tile([128, 2, d_out], F8)
        else:
            w1_sb = const.tile([128, nk, r], wdt)
            w2_sb = const.tile([128, d_out], wdt)
        nc.sync.dma_start(w1_sb[:], w1.ap())
        nc.sync.dma_start(w2_sb[:], w2.ap())
        s1_sb = const.tile([128, no], F32)
        nc.sync.dma_start(s1_sb[:], s1c.ap())
        b_sb = const.tile([128, no], F32)
        nc.sync.dma_start(b_sb[:], biasc.ap())

        if loop > 1:
            loop_cm = tc.For_i(
                0, loop, 1,
                hint_engines=(mybir.EngineType.PE, mybir.EngineType.DVE,
                              mybir.EngineType.Activation,
                              mybir.EngineType.SP))
            ctx.enter_context(loop_cm)

        for t in range(nt):
            z1p = z1pool.tile([128, t_tile], F32)
            xg, xg2 = {}, {}
            for kg in range(nk // g):
                if mode == "fp8dr":
                    xk = xpool.tile([128, g, 2, t_tile], F8)
                    if probe != "nodma":
                        in_dma().dma_start(
                            xk[:], xt.ap()[t, kg * g:(kg + 1) * g].rearrange(
                                "g p two s -> p g two s"))
                    xg[kg] = xk
                    continue
                xk = xpool.tile([128, g, t_tile], xdt)
                if probe != "nodma":
                    if layout == "fat":
                        in_dma().dma_start(xk[:], xt.ap()[t, kg])
                    else:
                        in_dma().dma_start(
                            xk[:], xt.ap()[t, kg * g:(kg + 1) * g].rearrange(
                                "g p s -> p g s"))
                xg[kg] = xk
                if mode == "bf16x2h":
                    xk2 = xpool.tile([128, g, t_tile], BF16, tag="xk2",
                                     name="xk2")
                    if probe != "nodma":
                        in_dma().dma_start(
                            xk2[:],
                            xt2.ap()[t, kg * g:(kg + 1) * g].rearrange(
                                "g p s -> p g s"))
                    xg2[kg] = xk2
            for k in range(nk):
                first, last = k == 0, k == nk - 1
                if mode == "fp8dr":
                    xk = xg[k // g][:, k % g, :, :]
                    nc.tensor.matmul(z1p[:], w1_sb[:, k, :, :], xk,
                                     start=first, stop=last, perf_mode=DR)
                    continue
                xk = xg[k // g][:, k % g, :]
                if mode == "bf16x2h":
                    xk2 = xg2[k // g][:, k % g, :]
                    nc.tensor.matmul(z1p[:], w1_sb[:, k, :], xk,
                                     start=first, stop=False)
                    nc.tensor.matmul(z1p[:], w1_sb[:, k, :], xk2,
                                     start=False, stop=last)
                elif mode in ("bf16", "bf16x2"):
                    hi = hpool.tile([128, t_tile], BF16)
                    nc.scalar.activation(hi[:], xk, Copy)
                    if mode == "bf16x2":
                        lo = lpool.tile([128, t_tile], BF16)
                        lo_iface.tensor_tensor(lo[:], xk, hi[:], sub)
                        nc.tensor.matmul(z1p[:], w1_sb[:, k, :], hi[:],
                                         start=first, stop=False)
                        nc.tensor.matmul(z1p[:], w1_sb[:, k, :], lo[:],
                                         start=False, stop=last)
                    else:
                        nc.tensor.matmul(z1p[:], w1_sb[:, k, :], hi[:],
                                         start=first, stop=last)
                else:
                    nc.tensor.matmul(z1p[:], w1_sb[:, k, :], xk,
                                     start=first, stop=last)

            z1hl = None
            if mode == "fp8dr":
                z1hl = z1s.tile([128, 2, t_tile], F8, tag="z1hl")
                nc.vector.tensor_copy(z1hl[:, 0, :], z1p[:])
                nc.vector.tensor_tensor(z1hl[:, 1, :], z1p[:], z1hl[:, 0, :],
                                        sub)
                movers = []
            elif mode in ("bf16", "bf16x2", "bf16x2h", "bf16s"):
                z1hi = z1s.tile([128, t_tile], BF16, tag="z1hi")
                nc.vector.tensor_copy(z1hi[:], z1p[:])
                movers = [z1hi]
                if mode in ("bf16x2", "bf16x2h"):
                    z1lo = z1s.tile([128, t_tile], BF16, tag="z1lo")
                    nc.vector.tensor_tensor(z1lo[:], z1p[:], z1hi[:], sub)
                    movers.append(z1lo)
            else:
                z1f = z1s.tile([128, t_tile], xdt, tag="z1f")
                nc.vector.tensor_copy(z1f[:], z1p[:])
                movers = [z1f]

            for og in range(no // g):
                ob = osb.tile([128, g, t_tile], out_dt)
                for oi in range(g):
                    o = og * g + oi
                    op = opsum.tile([128, t_tile], F32)
                    if mode == "fp8dr":
                        nc.tensor.matmul(
                            op[:], w2_sb[:, :, o * 128:(o + 1) * 128],
                            z1hl[:], start=True, stop=True, perf_mode=DR)
                    else:
                        for i, mv in enumerate(movers):
                            nc.tensor.matmul(
                                op[:], w2_sb[:, o * 128:(o + 1) * 128], mv[:],
                                start=(i == 0), stop=(i == len(movers) - 1))
                    if epi == "act":
                        nc.scalar.activation(ob[:, oi, :], op[:], Ident,
                                             bias=b_sb[:, o:o + 1],
                                             scale=s1_sb[:, o:o + 1])
                    else:
                        nc.vector.tensor_scalar(ob[:, oi, :], op[:],
                                                s1_sb[:, o:o + 1],
                                                b_sb[:, o:o + 1], mult, add)
                if probe != "nodma":
                    if layout == "fat":
                        out_dma().dma_start(outt.ap()[t, og], ob[:])
                    else:
                        out_dma().dma_start(
                            outt.ap()[t, og * g:(og + 1) * g].rearrange(
                                "g p s -> p g s"), ob[:])

    nc.compile()
    return nc


def prep_inputs(x, U_latent, V_latent, s1, s2, bias, mode=MODE,
                n_cores=N_CORES, t_tile=T_TILE, layout=LAYOUT,
                dma_group=DMA_GROUP):
    """Host-side prep: fold s2 into x, sign + cast factors, shard tokens."""
    import ml_dtypes

    tokens = x.shape[0] * x.shape[1] if x.ndim == 3 else x.shape[0]
    d_in = x.shape[-1]
    tok_pc = tokens // n_cores
    nt, nk = tok_pc // t_tile, d_in // 128
    g = dma_group

    x2 = x.reshape(tokens, d_in) * s2[None, :]
    w1 = np.sign(V_latent).astype(np.float32)
    # pack [d_in, r] -> [128, nk, r] so the SBUF upload is contiguous
    w1 = np.ascontiguousarray(
        w1.reshape(nk, 128, -1).transpose(1, 0, 2))
    w2 = np.ascontiguousarray(np.sign(U_latent).astype(np.float32).T)
    if mode in ("bf16", "bf16x2", "bf16x2h", "bf16s"):
        w1 = w1.astype(ml_dtypes.bfloat16)
        w2 = w2.astype(ml_dtypes.bfloat16)
    elif mode == "fp8dr":
        f8 = ml_dtypes.float8_e4m3
        # duplicate each sign chunk into both DoubleRow k-tile slots
        w1 = np.ascontiguousarray(
            np.stack([w1, w1], axis=2)).astype(f8)  # [128, nk, 2, r]
        w2 = np.ascontiguousarray(
            np.stack([w2, w2], axis=1)).astype(f8)  # [r, 2, d_out]
    if mode == "bf16x2h":
        xhi = x2.astype(ml_dtypes.bfloat16)
        xlo = (x2 - xhi.astype(np.float32)).astype(ml_dtypes.bfloat16)
    elif mode == "bf16s":
        x2 = x2.astype(ml_dtypes.bfloat16)
    elif mode == "fp8dr":
        f8 = ml_dtypes.float8_e4m3
        xhi = x2.astype(f8)
        xlo = (x2 - xhi.astype(np.float32)).astype(f8)
    no = w2.shape[1] // 128
    s1c = np.ascontiguousarray(s1.reshape(no, 128).T)
    biasc = np.ascontiguousarray(bias.reshape(no, 128).T)

    def tilefmt(arr2d, c):
        xs = arr2d[c * tok_pc:(c + 1) * tok_pc, :]
        if layout == "fat":
            # [nt, T, nk/g, g, 128] -> [nt, nk/g, 128, g, T]:
            # per partition a contiguous g*T run
            return np.ascontiguousarray(
                xs.reshape(nt, t_tile, nk // g, g, 128).transpose(
                    0, 2, 4, 3, 1))
        # [nt, T, nk, 128] -> [nt, nk, 128, T]
        return np.ascontiguousarray(
            xs.reshape(nt, t_tile, nk, 128).transpose(0, 2, 3, 1))

    in_maps = []
    for c in range(n_cores):
        m = {"w1": w1, "w2": w2, "s1c": s1c, "biasc": biasc}
        if mode == "bf16x2h":
            m["xt"] = tilefmt(xhi, c)
            m["xt2"] = tilefmt(xlo, c)
        elif mode == "fp8dr":
            # [nt, nk, 128, 2, T]: hi/lo interleaved per k-chunk
            m["xt"] = np.ascontiguousarray(
                np.stack([tilefmt(xhi, c), tilefmt(xlo, c)], axis=3))
        else:
            m["xt"] = tilefmt(x2, c)
        in_maps.append(m)
    return in_maps


def gather_out(results, n_cores=N_CORES, t_tile=T_TILE, layout=LAYOUT,
               dma_group=DMA_GROUP):
    out = np.empty((TOKENS, D_OUT), np.float32)
    for c in range(n_cores):
        ot = results[c]["outt"]
        if ot.dtype != np.float32:
            ot = ot.astype(np.float32)
        if layout == "fat":
            # [nt, no/g, 128, g, T] -> [tok_pc, d_out]
            shard = ot.transpose(0, 4, 1, 3, 2).reshape(TOK_PER_CORE, D_OUT)
        else:
            # [nt, no, 128, T] -> [tok_pc, d_out]
            shard = ot.transpose(0, 3, 1, 2).reshape(TOK_PER_CORE, D_OUT)
        out[c * TOK_PER_CORE:(c + 1) * TOK_PER_CORE, :] = shard
    return out.reshape(B, S, D_OUT)


_NC_CACHE = {}


def run(inputs, mode=MODE, trace=False):
    if mode not in _NC_CACHE:
        _NC_CACHE[mode] = build_nc(mode=mode)
    nc = _NC_CACHE[mode]
    in_maps = prep_inputs(**inputs, mode=mode)
    res = run_bass_kernel_spmd(nc, in_maps, list(range(N_CORES)),
                               trace=trace)
    return gather_out(res.results), res


def kernel(**inputs):
    inputs = {k: np.asarray(v) for k, v in inputs.items()}
    out, _ = run(inputs)
    return out

